# revision 79
# baseline (speedup 1.0000x reference)
"""Trainium2 Bass kernel for nn_Aggregate (gnn_message_passing).

Sharding: 8 cores = 2 directions x 4 batch-groups. Cores 0-3 compute
refined_async (source = sync_fea, adj = sync_adj, weights a_*) for 8
batches each; cores 4-7 compute refined_sync. The feature passthrough
(output channels 512:1024) and the no-neighbor fallback select are pure
input data movement, done host-side during unsharding.

Device algorithm per core (8 batches, one direction):
  Activations stay feature-major ([feat, node]); batches pair-stacked on
  partitions (rows 0-47 / 64-111) for the per-(batch,head) 48x48 blocks.

  Projections and the output map run as fp8e4 DoubleRow matmuls (0.5
  cycles/col, the full 256-deep contraction in one instruction): x and
  all weights are stored [128, 2, *] with the k-chunk in dim 1, weights
  host-prescaled by 32 (64 for Wm@Wo) to sit in e4m3's mantissa sweet
  spot; the descale rides existing eviction scale slots.

  Two exact algebraic folds shrink both data and compute:
   - bk is dropped: q^T bk and bq^T bk are per-query-constant in the
     softmax over keys, so they cancel; only (x Wq + bq)^T (x Wk) is
     needed.
   - bv is folded: sum_k SmT_h[k,t] = cnt[t] for every head, so the
     v-bias contributes (Wm(Wo bv))*r to the output; it merges with the
     existing (Wm bo)*r term into c0 = Wm(Wo bv + bo).

    qT = (1/32)*q_psum + s*bq   (Act)     kT = (1/32)*k_psum  (DVE)
    per (batch, head):  Pq = exp(qT_h^T kT_h), Pk = exp(kT_h^T qT_h)
                        den = Pk^T Af ; rec = 1/den ; w = Af * rec
                        ST  = Pq^T w ; SmT = ST * Af ; G_h = v_h^T SmT
    G' = gp * (16 r^2)  [fp8]  ;  M2 = (64 WmWo)^T G'  (DoubleRow)
    out = M2*(1/1024) + (c0*r + bm)

Built on bacc.Bacc: its compile() legalizes sync waits (TRN2 allows one
wait per instruction) via ldweights-wait motion + event semaphores.
"""

import numpy as np

FEA, H, B, N = 256, 8, 32, 48
DH = FEA // H
NB = 8            # batches per core
NPAIR = NB // 2
NCORES = 8
NT = NB * N       # 384

WS = 32.0         # fp8 prescale for Wq/Wk/Wv
WOMS = 64.0       # fp8 prescale for Wm@Wo
ALPHA = 16.0      # G' = gp * r^2 * ALPHA; out = m2/(WOMS*ALPHA) + rc

_cached = None


class _Stop(Exception):
    pass


def _build_program(phase_limit=99):
    import concourse.tile as tile
    from concourse.tile import add_dep_helper
    from concourse import bacc, mybir
    from contextlib import ExitStack

    f32 = mybir.dt.float32
    bf = mybir.dt.bfloat16
    f8 = mybir.dt.float8e4
    AF = mybir.ActivationFunctionType
    OP = mybir.AluOpType
    DR = mybir.MatmulPerfMode.DoubleRow

    nc = bacc.Bacc("TRN2", target_bir_lowering=False, debug=False)

    # ---- DRAM I/O ----
    hot1_d = nc.dram_tensor("hot1", [128, 2, NT + FEA], f8, kind="ExternalInput")
    hot2_d = nc.dram_tensor("hot2", [128, 2, 384], f8, kind="ExternalInput")
    hot3_d = nc.dram_tensor("hot3", [128, 2, FEA], f8, kind="ExternalInput")
    wom_d = nc.dram_tensor("wom", [128, 2, FEA], f8, kind="ExternalInput")
    adjt_d = nc.dram_tensor("adjt", [128, 1424], bf, kind="ExternalInput")
    out_d = nc.dram_tensor("outT", [128, 2 * NT], bf, kind="ExternalOutput")

    with ExitStack() as ctx:
      try:
        tc = ctx.enter_context(tile.TileContext(nc))
        sb = ctx.enter_context(tc.tile_pool(name="sb", bufs=1))
        ps = ctx.enter_context(tc.tile_pool(name="ps", bufs=4, space="PSUM"))

        # ---- loads: all on the SP queue in need order (x+Wk first: its
        # eviction gates the first scores); the q bias rides hot2 as raw
        # bytes so nothing early waits on the big adjacency blob ----
        hot1 = sb.tile([128, 2, NT + FEA], f8, tag="hot1")
        nc.sync.dma_start(out=hot1[:, :, :], in_=hot1_d.ap()[:, :, :])
        hot2 = sb.tile([128, 2, 384], f8, tag="hot2")
        nc.sync.dma_start(out=hot2[:, :, :], in_=hot2_d.ap()[:, :, :])
        hot3 = sb.tile([128, 2, FEA], f8, tag="hot3")
        nc.sync.dma_start(out=hot3[:, :, :], in_=hot3_d.ap()[:, :, :])
        adjt = sb.tile([128, 1424], bf, tag="adjt")
        nc.sync.dma_start(out=adjt[:, :], in_=adjt_d.ap()[:, :])
        wom = sb.tile([128, 2, FEA], f8, tag="wom")
        nc.sync.dma_start(out=wom[:, :, :], in_=wom_d.ap()[:, :, :])

        xT = hot1[:, :, 0:NT]
        wk = hot1[:, :, NT:NT + FEA]
        wq = hot2[:, :, 0:FEA]
        wv = hot3[:, :, 0:FEA]
        adjst = adjt[:, 0:NPAIR * N]
        # two fp32 consts (q bias per ot) ride hot2 as raw bytes
        bqs = hot2[:, 0, FEA:FEA + 8].bitcast(f32)
        adjst2 = adjt[:, 208:400]       # Af * 16*r^2 (SmT mask, r2 folded)
        rone = adjt[0:2, 400:784]       # row 0 = r, row 1 = ones
        cbm = adjt[0:2, 784:1040]       # rows: 1024*c0 | 1024*bm
        rc0 = adjt[:, 1040:1424]        # c0*r + bm, ot0 chunk (fo0 fma)

        # A dummy 1-col activation right at the top anchors the implicit
        # LoadActFuncSet (1.3us) into the DMA dead time, instead of letting
        # it land in front of the first real eviction.
        warm = sb.tile([128, 1], bf, tag="warm")
        nc.gpsimd.memset(warm[:, :], 0.0)
        nc.scalar.activation(out=warm[:, :], in_=warm[:, :], func=AF.Exp)

        _psn = [0]

        def pstile():
            _psn[0] += 1
            return ps.tile([128, 2, 512], f32, tag="ps", name=f"ps{_psn[0]}")

        # ---- q/k projections: fp8 DoubleRow, full 256-contraction per mm.
        # rhs free is kept <= 512 (2 k-tiles x 192), so 4 mm per tensor.
        # k runs first: its (single-op) eviction gates the first scores. ----
        pk, pq = pstile(), pstile()
        for p, w_ in ((pk, wk), (pq, wq)):
            for ot in range(2):
                for hf in range(2):
                    nc.tensor.matmul(
                        p[:, ot, hf * 192:(hf + 1) * 192],
                        w_[:, :, ot * 128:(ot + 1) * 128],
                        xT[:, :, hf * 192:(hf + 1) * 192],
                        start=True, stop=True, perf_mode=DR,
                    )

        # ---- evictions: kT per-ot on DVE, q on Act (scale + per-ot bias).
        # qT/kT gate the scores -> exp chain; explicit edges below pin the
        # v evictions behind them so the scheduler cannot reorder. ----
        kT = sb.tile([128, 2, NT], bf, tag="kT")
        qT = sb.tile([128, 2, NT], bf, tag="qT")
        kev, qev = [], []
        with nc.allow_low_precision(reason="bf16 activations"):
            for ot in range(2):
                kev.append(nc.vector.tensor_scalar_mul(
                    out=kT[:, ot, :], in0=pk[:, ot, 0:NT], scalar1=1.0 / WS,
                ))
            for ot in range(2):
                qev.append(nc.scalar.activation(
                    out=qT[:, ot, :], in_=pq[:, ot, 0:NT], func=AF.Identity,
                    scale=1.0 / WS, bias=bqs[:, ot:ot + 1],
                ))

        def head_slice(t, h, b):
            """[32, 48] slice of a feature-major [128, 2, NT] tile."""
            return t[(h % 4) * 32:(h % 4) * 32 + 32, h // 4, b * N:(b + 1) * N]

        # ---- scores, both orientations, exp ----
        # Row-tiled matmuls must not write the same PSUM bank concurrently
        # (HW constraint). Map row-group -> bank bijectively: tile t's bank b
        # holds head-group g = 2t+b (heads {g, g+4}), pairs in the free dim.
        if phase_limit < 3:
            _finish(nc, out_d, qT); raise _Stop

        def p_off(pr, h):
            return (h % 4) * 384 + (h // 4) * 192 + pr * 48

        score_mms = {"k": [], "q": []}

        def scores_tile(orient, t, dst):
            lhs, rhs = (qT, kT) if orient == "q" else (kT, qT)
            p = pstile()
            for b_ in range(2):
                g = 2 * t + b_
                for hh in range(2):
                    h = hh * 4 + g
                    for pr in range(NPAIR):
                        for j in range(2):
                            bb = pr * 2 + j
                            score_mms[orient].append(nc.tensor.matmul(
                                p[j * 64:j * 64 + N, b_,
                                  hh * 192 + pr * 48:hh * 192 + pr * 48 + N],
                                head_slice(lhs, h, bb),
                                head_slice(rhs, h, bb),
                                start=True, stop=True,
                                tile_position=(g * 32, j * 64),
                            ))
            nc.scalar.activation(
                out=dst[:, 2 * t * 384:(2 * t + 2) * 384]
                    .rearrange("p (b f) -> p b f", f=384),
                in_=p[:, :, 0:384], func=AF.Exp,
            )

        def scores_block(orient):
            dst = sb.tile([128, 4 * 384], bf, tag=f"P{orient}",
                          name=f"P{orient}")
            for t in range(2):
                scores_tile(orient, t, dst)
            return dst

        Pk = scores_block("k")

        # ---- v: fp8 DoubleRow. DoubleRow dst must sit at partition 0, so
        # all 8 batches land on rows 0-47 (batch b -> tile b//4, bank
        # (b//2)%2, off (b%2)*FEA); the j=1 half is row-shifted later by an
        # SBUF->SBUF DMA on the idle Pool queue. ----
        pvt = [pstile(), pstile()]
        v_mms = []
        for b in range(NB):
            v_mms.append(nc.tensor.matmul(
                pvt[b // 4][0:N, (b // 2) % 2, (b % 2) * FEA:(b % 2) * FEA + FEA],
                xT[:, :, b * N:(b + 1) * N],
                wv[:, :, :],
                start=True, stop=True, perf_mode=DR,
            ))


        Pq = sb.tile([128, 4 * 384], bf, tag="Pq", name="Pq")
        scores_tile("q", 0, Pq)

        # ---- den + w ----
        # Paired psum tiles (bank = pr parity) keep the 3-slot rotation fed;
        # within a bank the two j-halves carry a sync edge (row-tiled writes
        # to one bank must not overlap in time).
        if phase_limit < 4:
            _finish(nc, out_d, Pq[:, 0:768].rearrange("p (a f) -> p a f", f=384))
            raise _Stop
        wT = sb.tile([128, NPAIR * 384], bf, tag="wT")
        SmT = sb.tile([128, NPAIR * 384], bf, tag="SmT")
        Sb = sb.tile([128, 4, 384], bf, tag="Sb")

        def adj_qslice(pr):
            """Af[part, h(bcast), t] for one pair."""
            return adjst[:, pr * N:(pr + 1) * N][:, None, :] \
                .to_broadcast((128, H, N))

        def adj2_qslice(pr):
            """Af*16r^2 [part, h(bcast), t] for one pair (SmT mask)."""
            return adjst2[:, pr * N:(pr + 1) * N][:, None, :] \
                .to_broadcast((128, H, N))

        def serial_rowgroups(mms_j0, mms_j1):
            for i1 in mms_j1:
                for i0 in mms_j0:
                    add_dep_helper(i1.ins, i0.ins, sync=True,
                                   reason="same-bank row-group serialization")

        dtile = {}

        def den_pair(pr):
            dp = dtile[pr // 2]
            groups = [[], []]
            for j in range(2):
                for h in range(H):
                    groups[j].append(nc.tensor.matmul(
                        dp[j * 64:j * 64 + N, pr % 2, h * N:(h + 1) * N],
                        Pk[j * 64:j * 64 + N, p_off(pr, h):p_off(pr, h) + N],
                        adjst[j * 64:j * 64 + N, pr * N:(pr + 1) * N],
                        start=True, stop=True,
                    ))
            serial_rowgroups(groups[0], groups[1])
            rec = sb.tile([128, 384], bf, tag=f"rec{pr}", name=f"rec{pr}")
            with nc.allow_low_precision(reason="bf16 attn weights; psum accum stays fp32"):
                nc.vector.reciprocal(out=rec[:, :], in_=dp[:, pr % 2, 0:384])
            # all-bf16 all-SBUF: runs in the DVE 2x fast path
            nc.vector.tensor_tensor(
                out=wT[:, pr * 384:(pr + 1) * 384]
                    .rearrange("p (h t) -> p h t", t=N),
                in0=adj_qslice(pr),
                in1=rec[:, :].rearrange("p (h t) -> p h t", t=N),
                op=OP.mult,
            )

        scores_tile("q", 1, Pq)
        # batch b mapping: tile t holds batches 4t..4t+3; j0 batches (b even)
        # sit at off 0 covering (bank=pr//2=t, c=pr%2); j1 at off FEA.
        # One eviction per pvt tile (both halves -> vall rows 0-47) chained on
        # DVE behind the kT evictions so each pvt slot recycles promptly for
        # the scores-q psum tiles. The j1 half then row-shifts to vj1
        # via a Pool-queue DMA (already descaled, plain byte copy).
        vall = sb.tile([N, 2, 2, 2, FEA], bf, tag="vall")   # [n, j, t, c, o]
        vj1 = sb.tile([128, 2, 2, FEA], bf, tag="vj1")
        with nc.allow_low_precision(reason="bf16 activations"):
            vd = [nc.vector.tensor_scalar_mul(
                out=vall[:, :, t, :, :].rearrange("n j c o -> n c j o"),
                in0=pvt[t][0:N, :, :].rearrange("p a (c o) -> p a c o", o=FEA),
                scalar1=1.0 / WS,
            ) for t in range(2)]
        prev = kev[-1]
        for later in vd:
            add_dep_helper(later.ins, prev.ins, sync=True,
                           reason="DVE eviction chain: kT first, then pvt0, pvt1")
            prev = later
        nc.gpsimd.dma_start(
            out=vj1[64:64 + N, :, :, :].rearrange("p t c o -> p (t c o)"),
            in_=vall[:, 1, :, :, :].rearrange("n t c o -> n (t c o)"))

        dtile[0] = pstile()
        den_pair(0)
        den_pair(1)
        dtile[1] = pstile()
        den_pair(2)
        den_pair(3)
        if phase_limit < 5:
            _finish(nc, out_d, wT[:, 0:768].rearrange("p (a f) -> p a f", f=384))
            raise _Stop

        # ---- S + SmT. SmT carries the host-folded Af*16r^2 mask, so G
        # leaves the PE already r^2-scaled. Pairs 0,2 stage through an Act
        # copy (DVE does only the cheap bf16 mask); 1,3 go direct on DVE. ----
        stile = {0: pstile(), 1: pstile()}   # S01, S23
        for pr in range(NPAIR):
            sp = stile[pr // 2]
            groups = [[], []]
            for j in range(2):
                for h in range(H):
                    groups[j].append(nc.tensor.matmul(
                        sp[j * 64:j * 64 + N, pr % 2, h * N:(h + 1) * N],
                        Pq[j * 64:j * 64 + N, p_off(pr, h):p_off(pr, h) + N],
                        wT[j * 64:j * 64 + N,
                           pr * 384 + h * N:pr * 384 + h * N + N],
                        start=True, stop=True,
                    ))
            serial_rowgroups(groups[0], groups[1])
            dst = SmT[:, pr * 384:(pr + 1) * 384].rearrange("p (h t) -> p h t", t=N)
            with nc.allow_low_precision(reason="bf16 attn sums"):
                if pr % 2 == 0:
                    nc.scalar.activation(
                        out=Sb[:, pr // 2, :], in_=sp[:, pr % 2, 0:384],
                        func=AF.Copy,
                    )
                    nc.vector.tensor_tensor(
                        out=dst,
                        in0=Sb[:, pr // 2, :].rearrange("p (h t) -> p h t", t=N),
                        in1=adj2_qslice(pr), op=OP.mult,
                    )
                else:
                    nc.vector.tensor_tensor(
                        out=dst,
                        in0=sp[:, pr % 2, 0:384].rearrange("p (h t) -> p h t", t=N),
                        in1=adj2_qslice(pr), op=OP.mult,
                    )

        # ---- G: pooled-pre, feature-major; bank = batch parity (= row grp j)
        if phase_limit < 6:
            _finish(nc, out_d, SmT[:, 0:768].rearrange("p (a f) -> p a f", f=384))
            raise _Stop
        gp = pstile()
        m2 = pstile()
        G = sb.tile([128, 2, NT], f8, tag="G")

        def g_mms(b2):
            for j in range(2):
                bb = b2 * 2 + j
                pr = bb // 2
                for h in range(H):
                    vsrc = (vall[0:N, 0, pr // 2, pr % 2, h * 32:(h + 1) * 32]
                            if j == 0 else
                            vj1[64:64 + N, pr // 2, pr % 2, h * 32:(h + 1) * 32])
                    nc.tensor.matmul(
                        gp[(h % 4) * 32:(h % 4) * 32 + 32, j,
                           (h // 4) * 192 + b2 * 48:(h // 4) * 192 + b2 * 48 + N],
                        vsrc,
                        SmT[j * 64:j * 64 + N,
                            pr * 384 + h * N:pr * 384 + h * N + N],
                        start=True, stop=True,
                        tile_position=(j * 64, (h % 4) * 32),
                    )

        for b2 in range(NPAIR):
            g_mms(b2)
        # r^2 already folded via the SmT mask: eviction is one plain copy
        with nc.allow_low_precision(reason="fp8 G; error repaid in 256-contraction"):
            nc.scalar.activation(
                out=G[:, :, :].rearrange("p c (b2 j n) -> p c b2 j n",
                                         b2=NPAIR, j=2),
                in_=gp[:, :, 0:384].rearrange("p j (c b2 n) -> p c b2 j n",
                                              c=2, b2=NPAIR),
                func=AF.Copy,
            )
        for ot in (1, 0):
            for hf in range(2):
                nc.tensor.matmul(
                    m2[:, ot, hf * 192:(hf + 1) * 192],
                    wom[:, :, ot * 128:(ot + 1) * 128],
                    G[:, :, hf * 192:(hf + 1) * 192],
                    start=True, stop=(ot == 0), perf_mode=DR,
                )
                if ot == 1:
                    nc.tensor.matmul(
                        m2[:, ot, hf * 192:(hf + 1) * 192],
                        cbm[:, ot * 128:(ot + 1) * 128],
                        rone[:, hf * 192:(hf + 1) * 192],
                        start=False, stop=True, skip_group_check=True,
                    )

        if phase_limit < 7:
            _finish(nc, out_d, G); raise _Stop

        # ---- tail: out = m2/1024, ot0 on DVE, ot1 on Act, DMA per ot ----
        osb = sb.tile([128, 2, NT], bf, tag="osb")
        with nc.allow_low_precision(reason="bf16 output"):
            nc.vector.tensor_scalar_mul(
                out=osb[:, 1, :], in0=m2[:, 1, 0:NT],
                scalar1=1.0 / (WOMS * ALPHA),
            )
            nc.vector.scalar_tensor_tensor(
                out=osb[:, 0, :], in0=m2[:, 0, 0:NT],
                scalar=1.0 / (WOMS * ALPHA), in1=rc0[:, :],
                op0=OP.mult, op1=OP.add,
            )
        nc.sync.dma_start(out=out_d.ap()[:, 0:NT], in_=osb[:, 0, :])
        nc.scalar.dma_start(out=out_d.ap()[:, NT:2 * NT], in_=osb[:, 1, :])
      except _Stop:
        pass

    nc.compile()
    return nc


def _finish(nc, out_d, tile_ap):
    ap = tile_ap[:, :, :].rearrange("p a t -> p (a t)")
    for f0 in range(0, 2 * NT, 96):
        nc.sync.dma_start(out=out_d.ap()[:, f0:f0 + 96], in_=ap[:, f0:f0 + 96])


def _get_program():
    global _cached
    if _cached is None:
        _cached = _build_program()
    return _cached


def _prep_core_inputs(x_src, adj, Wq, bq, Wk, bk, Wv, bv, Wo, bo, Wm, bm):
    """Host-side shard prep for one core: 8 batches of one direction.
    Matmul-side tensors go to fp8e4 (weights prescaled into e4m3's sweet
    spot; descales folded into on-chip eviction constants)."""
    import ml_dtypes
    f32 = np.float32
    bf = ml_dtypes.bfloat16
    f8 = ml_dtypes.float8_e4m3
    s = 1.0 / np.sqrt(np.float32(DH))

    xT = np.transpose(x_src, (2, 0, 1)).reshape(FEA, NT)
    hot1 = np.zeros((128, 2, NT + FEA), f32)
    hot2 = np.zeros((128, 2, 384), f32)
    hot3 = np.zeros((128, 2, FEA), f32)
    for kc in range(2):
        rows = slice(kc * 128, (kc + 1) * 128)
        hot1[:, kc, 0:NT] = xT[rows]
        hot1[:, kc, NT:NT + FEA] = WS * Wk.T[rows]
        hot2[:, kc, 0:FEA] = (WS * s) * Wq.T[rows]
        hot3[:, kc, 0:FEA] = WS * Wv.T[rows]
    hot1 = hot1.astype(f8)
    hot2 = hot2.astype(f8)
    hot3 = hot3.astype(f8)
    bqs = np.zeros((128, 2), np.float32)
    bqs[:, 0:2] = (bq * s).reshape(2, 128).T
    hot2.view(np.uint8)[:, 0, FEA:FEA + 8] = bqs.view(np.uint8)

    womT = (Wm @ Wo).T
    wom = np.zeros((128, 2, FEA), f32)
    for kc in range(2):
        wom[:, kc, :] = WOMS * womT[kc * 128:(kc + 1) * 128]

    Af = (adj > 0).astype(f32)                       # [NB, 48(k), 48(t)]
    cnt = Af.sum(axis=1)                             # [NB, 48(t)]
    r = 1.0 / np.maximum(cnt, 1.0)
    r2s = ALPHA * r * r                              # folded into the SmT mask
    adjt = np.zeros((128, 1424), f32)
    for p in range(NPAIR):
        adjt[0:N, p * N:(p + 1) * N] = Af[2 * p]
        adjt[64:64 + N, p * N:(p + 1) * N] = Af[2 * p + 1]
        adjt[0:N, 208 + p * N:208 + (p + 1) * N] = Af[2 * p] * r2s[2 * p]
        adjt[64:64 + N, 208 + p * N:208 + (p + 1) * N] = Af[2 * p + 1] * r2s[2 * p + 1]
    adjt[0, 400:784] = r.reshape(NT)
    adjt[1, 400:784] = 1.0
    c0 = (Wm @ (Wo @ bv + bo)).astype(f32)
    adjt[0, 784:1040] = (WOMS * ALPHA) * c0
    adjt[1, 784:1040] = (WOMS * ALPHA) * bm
    adjt[:, 1040:1424] = (np.outer(c0[0:128], r.reshape(NT))
                          + bm[0:128, None])
    adjt = adjt.astype(bf)
    return {
        "hot1": np.ascontiguousarray(hot1),
        "hot2": np.ascontiguousarray(hot2),
        "hot3": np.ascontiguousarray(hot3),
        "wom": np.ascontiguousarray(wom).astype(f8),
        "adjt": adjt,
    }


def _postprocess_core(out_dev, Af, fallback):
    """out_dev [128, 768] -> mapped [8, 48, 256]; apply fallback select."""
    arr = out_dev.reshape(128, 2, NB, N)
    mapped = np.ascontiguousarray(np.transpose(arr, (2, 3, 1, 0))).reshape(NB, N, FEA)
    cnt = Af.sum(axis=1)                              # [NB, 48(t)]
    return np.where((cnt > 0)[:, :, None], mapped, fallback)


def _make_in_maps(a):
    in_maps, meta = [], []
    for core in range(NCORES):
        dirn = "a" if core < 4 else "s"
        g = core % 4
        bs = slice(g * NB, (g + 1) * NB)
        if dirn == "a":
            x_src, adj, fb = a["sync_fea"][bs], a["sync_adj"][bs], a["async_fea"][bs]
        else:
            x_src, adj, fb = a["async_fea"][bs], a["async_adj"][bs], a["sync_fea"][bs]
        wkeys = [f"{dirn}_{w}" for w in
                 ("Wq", "bq", "Wk", "bk", "Wv", "bv", "Wo", "bo", "Wm", "bm")]
        in_maps.append(_prep_core_inputs(x_src, adj, *[a[k] for k in wkeys]))
        meta.append(((adj > 0).astype(np.float32), fb))
    return in_maps, meta


def _assemble(a, meta, results):
    out = np.zeros((B, N, 4 * FEA), np.float32)
    out[:, :, 2 * FEA:3 * FEA] = a["async_fea"]
    out[:, :, 3 * FEA:] = a["sync_fea"]
    for core in range(NCORES):
        Af, fb = meta[core]
        refined = _postprocess_core(results[core]["outT"], Af, fb)
        g = core % 4
        bs = slice(g * NB, (g + 1) * NB)
        col = slice(0, FEA) if core < 4 else slice(FEA, 2 * FEA)
        out[bs, :, col] = refined
    return out


def kernel(**inputs):
    from concourse import bass_utils

    nc = _get_program()
    a = {k: np.asarray(v) for k, v in inputs.items()}
    in_maps, meta = _make_in_maps(a)
    res = bass_utils.run_bass_kernel_spmd(nc, in_maps, core_ids=list(range(NCORES)))
    return _assemble(a, meta, res.results)
